# revision 4
# baseline (speedup 1.0000x reference)
"""Catmull-Rom activation kernel for 8 TRN2 NeuronCores.

Reference semantics (m=8192 samples, n=2048 neurons, K=10 control points):
  p0    = floor(((x+2)*6)/4 + 1), clamped to 1 at x<=-2 and 7 at x>=2
  u     = frac(2x)
  coef  = (U @ B)[:, ::-1]   with U = [u^3, u^2, u, 1]   (sample-major flat)
  Q_k   = CP[j, p0+k-1]                                  (neuron-major flat)
  out   = sum_k coef_k * Q_k elementwise ON MISMATCHED FLATTENS: at flat
          position p (sample-major), coef comes from x[p//n, p%n] while Q
          comes from neuron jq=p//m, sample iq=p%m.

Algebraic regrouping: out = ((H0*u + H1)*u + H2)*u + H3 where
  H_t = T_t[j, s],  T_t[j, v] = sum_k B[t, 3-k] * CP[j, v+k-1],  s in 1..7.

Per-core layout (core c of 8): all work happens in the neuron-major
"H layout" (256 neurons x 8192 samples). Each core receives
  xr  = x[1024c:1024(c+1), :].reshape(256, 8192)   (u side)
  xct = x[:, 256c:256(c+1)].T                       (segment side)
  cp  = control_points[256c:256(c+1), :]
and its (256, 8192) output block is exactly out rows [1024c, 1024(c+1))
reinterpreted. No collectives, no on-device transposes.

Compute structure (3 custom DVE ops, registered at import):
  CR_SEG:  d = ((clamp(x,-2,2)+2)*6)*0.25 + 1     1 inst (exact ref rounding)
  CR_FRAC: u = w + (w<0), w = 2x - rne(2x)        1 inst (magic-const round)
  CR_ACC2: acc + (d>=v)*s0 + (d>=v+1)*s1          3 insts per t (v=2,4,6)
           (s0/s1 are per-partition [P,1] table deltas; first call seeds
           the chain with the v=1 base via in1=[P,1] broadcast)
Horner runs on the Pool engine (tensor_tensor) to overlap with DVE.
"""

import sys

import numpy as np

sys.path.insert(0, "/opt/trn_rl_repo")

from contextlib import ExitStack

import concourse.bass as bass
import concourse.bacc as bacc
import concourse.mybir as mybir
from concourse import tile
from concourse import dve_ops
from concourse.dve_spec import (
    Spec, Src0, Src1, C0, C1, C2, Zero, One, maxx, minn, lower, _has_src1,
)
from concourse.dve_uop import DveOpSpec
from concourse.bass_utils import run_bass_kernel_spmd

M = 8192          # samples
N = 2048          # neurons
K = 10            # control points per neuron
NCORES = 8
NL = N // NCORES  # 256 neurons per core
P = 128           # partitions per tile
FT = 2048         # free-dim tile size
f32 = mybir.dt.float32
Alu = mybir.AluOpType
MAGIC = 12582912.0  # 1.5 * 2^23: rne-to-integer bias, valid for |t| < 2^22

# Wrev[t, k] = B[t, 3-k]; T_t[:, v] = sum_k Wrev[t,k] * CP[:, v-1+k]
_B = 0.5 * np.array(
    [[-1.0, 3.0, -3.0, 1.0],
     [2.0, -5.0, 4.0, -1.0],
     [-1.0, 0.0, 1.0, 0.0],
     [0.0, 2.0, 0.0, 0.0]], dtype=np.float32)
WREV = np.ascontiguousarray(_B[:, ::-1])  # (4, 4)

_CACHE = {}


def _register_op(name, spec):
    for o in dve_ops.OPS:
        if o.name == name:
            return o
    row = max(dve_ops._SUB_OPCODE_FOR_NAME.values()) + 1
    assert row < 0x20
    dve_ops._SUB_OPCODE_FOR_NAME[name] = row
    shas = {}
    for ver in ("v3", "v4"):
        u = lower(spec, ver=ver)
        shas[ver] = DveOpSpec(
            name=name, opcode=row, uops=u, rd1_en=_has_src1(spec)).sha(ver)
    op = dve_ops.DveOp(name, spec, subdim=False, uops_sha=shas)
    dve_ops.OPS.append(op)
    dve_ops.CUSTOM_DVE_SPECS[name] = spec
    return op


def _seg_ref(in0, in1, s0, s1, imm2):
    x = np.asarray(in0, np.float32)
    xcl = np.minimum(np.maximum(x, np.float32(-s1)), np.float32(s1))
    return (((xcl + np.float32(s1)) * np.float32(imm2)) * np.float32(s0)
            + np.float32(1.0)).astype(np.float32)


def _frac_ref(in0, in1, s0, s1, imm2):
    a = (np.asarray(in0, np.float32) * np.float32(s1)).astype(np.float32)
    r = ((a + np.float32(s0)).astype(np.float32) - np.float32(s0)).astype(np.float32)
    w = (a - r).astype(np.float32)
    return (w + (w < 0).astype(np.float32)).astype(np.float32)


def _acc2_ref(in0, in1, s0, s1, imm2):
    d = np.asarray(in0, np.float32)
    m0 = (d >= np.float32(imm2)).astype(np.float32)
    m1 = (d >= np.float32(imm2) + np.float32(1.0)).astype(np.float32)
    return (np.asarray(in1, np.float32) + m0 * np.asarray(s0, np.float32)
            + m1 * np.asarray(s1, np.float32)).astype(np.float32)


CR_SEG = _register_op("CR_SEG_ANT", Spec(
    body=((maxx(minn(Src0, C1), Zero - C1) + C1) * C2) * C0 + One,
    reference=_seg_ref))

_a = Src0 * C1
_w = _a - ((_a + C0) - C0)
CR_FRAC = _register_op("CR_FRAC_ANT", Spec(
    body=_w + (_w < Zero),
    reference=_frac_ref))

CR_ACC2 = _register_op("CR_ACC2_ANT", Spec(
    body=Src1 + (Src0 >= C2) * C0 + (Src0 >= (C2 + One)) * C1,
    reference=_acc2_ref))


def _init2_ref(in0, in1, s0, s1, imm2):
    # in1 is the C3-spilled [P,1] scalar (second delta); s1 = base
    d = np.asarray(in0, np.float32)
    m0 = (d >= np.float32(imm2)).astype(np.float32)
    m1 = (d >= np.float32(imm2) + np.float32(1.0)).astype(np.float32)
    return (np.asarray(s1, np.float32) + m0 * np.asarray(s0, np.float32)
            + m1 * np.asarray(in1, np.float32)).astype(np.float32)


from concourse.dve_spec import C3, _spill_c3_to_src1  # noqa: E402

CR_INIT2 = _register_op("CR_INIT2_ANT", Spec(
    body=_spill_c3_to_src1(
        C1 + (Src0 >= C2) * C0 + (Src0 >= (C2 + One)) * C3),
    reference=_init2_ref))


def _build_bass(gens: int = 1):
    nc = bacc.Bacc("TRN2", target_bir_lowering=False, debug=False,
                   num_devices=NCORES)
    xr = nc.dram_tensor("xr", [NL, M], f32, kind="ExternalInput").ap()
    xct = nc.dram_tensor("xct", [NL, M], f32, kind="ExternalInput").ap()
    cp = nc.dram_tensor("cp", [NL, K], f32, kind="ExternalInput").ap()
    out = nc.dram_tensor("out", [NL, M], f32, kind="ExternalOutput").ap()

    with tile.TileContext(nc, num_cores=NCORES) as tc, ExitStack() as ctx:
        const_pool = ctx.enter_context(tc.tile_pool(name="const", bufs=1))
        in_pool = ctx.enter_context(tc.tile_pool(name="inp", bufs=3))
        du_pool = ctx.enter_context(tc.tile_pool(name="du", bufs=2))
        acc_pool = ctx.enter_context(tc.tile_pool(name="acc", bufs=2))
        out_pool = ctx.enter_context(tc.tile_pool(name="outp", bufs=3))

        for gen in range(gens):
         for jb in range(NL // P):  # two 128-neuron blocks
            # ---- tiny per-block table prep (on DVE; negligible) ----
            cpt = const_pool.tile([P, K], f32, tag=f"cp{jb}")
            nc.sync.dma_start(cpt[:], cp[jb * P:(jb + 1) * P, :])
            # T_t[:, vi] for vi=0..6 (v=vi+1); D_t[:, vi] = T(vi+1)-T(vi)
            T = [const_pool.tile([P, 7], f32, tag=f"T{jb}_{t}", name=f"T{jb}_{t}")
                 for t in range(4)]
            D = [const_pool.tile([P, 6], f32, tag=f"D{jb}_{t}", name=f"D{jb}_{t}")
                 for t in range(4)]
            for t in range(4):
                nc.vector.tensor_single_scalar(
                    T[t][:], cpt[:, 0:7], float(WREV[t, 0]), Alu.mult)
                for k in range(1, 4):
                    nc.vector.scalar_tensor_tensor(
                        T[t][:], cpt[:, k:k + 7], float(WREV[t, k]), T[t][:],
                        Alu.mult, Alu.add)
                nc.vector.tensor_sub(D[t][:], T[t][:, 1:7], T[t][:, 0:6])

            for fc in range(M // FT):
                fsl = slice(fc * FT, (fc + 1) * FT)
                psl = slice(jb * P, (jb + 1) * P)

                xr_t = in_pool.tile([P, FT], f32, tag="xr")
                nc.sync.dma_start(xr_t[:], xr[psl, fsl])
                xc_t = in_pool.tile([P, FT], f32, tag="xc")
                nc.sync.dma_start(xc_t[:], xct[psl, fsl])

                # d on the otherwise-idle Scalar engine: d = 1.5*x + 4.
                # (no clamp needed: for x<=-2 no step fires, for x>=2 all do)
                d_t = du_pool.tile([P, FT], f32, tag="d")
                nc.scalar.activation(d_t[:], xc_t[:],
                                     mybir.ActivationFunctionType.Copy,
                                     bias=4.0, scale=1.5)
                # u on DVE (needs exact round-to-nearest via magic constant)
                u_t = du_pool.tile([P, FT], f32, tag="u")
                nc.vector._custom_dve(CR_FRAC, out=u_t[:], in0=xr_t[:],
                                      s0=MAGIC, s1=2.0)

                # H_t = T_t[:,1] + sum_{v=2..7} (d>=v) * D_t[:,v-2]; 3 insts/t
                # (INIT2: base via s1, delta v=2 via s0, v=3 via C3-spill).
                # Chains interleaved round-robin so consecutive DVE insts hit
                # different accumulators (no back-to-back RAW on one tile).
                ht = [acc_pool.tile([P, FT], f32, tag=f"h{t}", name=f"h{t}")
                      for t in range(4)]
                for t in range(4):
                    nc.vector._custom_dve(
                        CR_INIT2, out=ht[t][:], in0=d_t[:], in1=D[t][:, 1:2],
                        s0=D[t][:, 0:1], s1=T[t][:, 0:1], imm2=2.0)
                for t in range(4):
                    nc.vector._custom_dve(
                        CR_ACC2, out=ht[t][:], in0=d_t[:], in1=ht[t][:],
                        s0=D[t][:, 2:3], s1=D[t][:, 3:4], imm2=4.0)
                o_t = out_pool.tile([P, FT], f32, tag="o")
                for t in range(4):
                    nc.vector._custom_dve(
                        CR_ACC2, out=ht[t][:], in0=d_t[:], in1=ht[t][:],
                        s0=D[t][:, 4:5], s1=D[t][:, 5:6], imm2=6.0)
                    # Horner (Pool) fires as soon as each H_t completes
                    if t == 0:
                        nc.gpsimd.tensor_tensor(o_t[:], ht[0][:], u_t[:],
                                                Alu.mult)
                    else:
                        nc.gpsimd.tensor_tensor(o_t[:], o_t[:], ht[t][:],
                                                Alu.add)
                        if t < 3:
                            nc.gpsimd.tensor_tensor(o_t[:], o_t[:], u_t[:],
                                                    Alu.mult)

                nc.sync.dma_start(out[psl, fsl], o_t[:])

    nc.finalize()
    return nc


def _get_nc():
    if "nc" not in _CACHE:
        _CACHE["nc"] = _build_bass()
    return _CACHE["nc"]


def kernel(x: np.ndarray, control_points: np.ndarray) -> np.ndarray:
    x = np.ascontiguousarray(np.asarray(x, dtype=np.float32))
    cp = np.ascontiguousarray(np.asarray(control_points, dtype=np.float32))
    assert x.shape == (M, N) and cp.shape == (N, K)

    nc = _get_nc()
    mrows = M // NCORES  # 1024 output rows per core
    in_maps = []
    for c in range(NCORES):
        xr = np.ascontiguousarray(
            x[c * mrows:(c + 1) * mrows, :]).reshape(NL, M)
        xct = np.ascontiguousarray(x[:, c * NL:(c + 1) * NL].T)
        cpc = np.ascontiguousarray(cp[c * NL:(c + 1) * NL, :])
        in_maps.append({"xr": xr, "xct": xct, "cp": cpc})

    res = run_bass_kernel_spmd(nc, in_maps, core_ids=list(range(NCORES)))
    outs = [res.results[c]["out"].reshape(mrows, N) for c in range(NCORES)]
    return np.concatenate(outs, axis=0)



# revision 6
# speedup vs baseline: 1.3586x; 1.3586x over previous
"""Catmull-Rom activation kernel for 8 TRN2 NeuronCores.

Reference semantics (m=8192 samples, n=2048 neurons, K=10 control points):
  p0    = floor(((x+2)*6)/4 + 1), clamped to 1 at x<=-2 and 7 at x>=2
  u     = frac(2x)
  coef  = (U @ B)[:, ::-1]   with U = [u^3, u^2, u, 1]   (sample-major flat)
  Q_k   = CP[j, p0+k-1]                                  (neuron-major flat)
  out   = sum_k coef_k * Q_k elementwise ON MISMATCHED FLATTENS: at flat
          position p (sample-major), coef comes from x[p//n, p%n] while Q
          comes from neuron jq=p//m, sample iq=p%m.

Algebraic regrouping: out = H0*u^3 + H1*u^2 + H2*u + H3 where
  H_t = T_t[j, s],  T_t[j, v] = sum_k B[t, 3-k] * CP[j, v+k-1],  s in 1..7.

Per-core layout (core c of 8): all work happens in the neuron-major
"H layout" (256 neurons x 8192 samples). Each core receives
  xr  = x[1024c:1024(c+1), :].reshape(256, 8192)   (u side)
  xct = x[:, 256c:256(c+1)].T                       (segment side)
  cp  = control_points[256c:256(c+1), :]
and its (256, 8192) output block is exactly out rows [1024c, 1024(c+1))
reinterpreted. No collectives, no on-device transposes.

Engine assignment (GpSimd/Pool is deliberately IDLE: it shares an SBUF
port with the DVE and the two serialize, so any Pool op costs more port
time than the same op on DVE):
  Scalar/ACT (own SBUF port): d = 1.5*x+4 (segment line, steps saturate
      so no clamp needed); u^2 = Square(u); PSUM->SBUF output copy.
  Vector/DVE: u = frac(2x) via magic-rounding custom op (bf16 out);
      4 H_t step-accumulation chains (3 custom insts each, fp32,
      final inst writes bf16); 4 bf16 2x-mode tensor_tensor multiplies
      (h0*u^2, (h0*u^2)*u, h1*u^2, h2*u).
  Tensor/PE: final sum m0+m1+m2+h3 via identity-weight matmuls
      accumulated in PSUM (bf16 inputs, fp32 accumulate).
"""

import sys

import numpy as np

sys.path.insert(0, "/opt/trn_rl_repo")

from contextlib import ExitStack

import concourse.bass as bass
import concourse.bacc as bacc
import concourse.mybir as mybir
from concourse import tile
from concourse import dve_ops
from concourse.dve_spec import (
    Spec, Src0, Src1, C0, C1, C2, Zero, One, maxx, minn, lower, _has_src1,
)
from concourse.dve_uop import DveOpSpec
from concourse.bass_utils import run_bass_kernel_spmd

import ml_dtypes

M = 8192          # samples
N = 2048          # neurons
K = 10            # control points per neuron
NCORES = 8
NL = N // NCORES  # 256 neurons per core
P = 128           # partitions per tile
FT = 2048         # free-dim tile size
f32 = mybir.dt.float32
bf16 = mybir.dt.bfloat16
Alu = mybir.AluOpType
Act = mybir.ActivationFunctionType
MAGIC = 12582912.0  # 1.5 * 2^23: rne-to-integer bias, valid for |t| < 2^22

# Wrev[t, k] = B[t, 3-k]; T_t[:, v] = sum_k Wrev[t,k] * CP[:, v-1+k]
_B = 0.5 * np.array(
    [[-1.0, 3.0, -3.0, 1.0],
     [2.0, -5.0, 4.0, -1.0],
     [-1.0, 0.0, 1.0, 0.0],
     [0.0, 2.0, 0.0, 0.0]], dtype=np.float32)
WREV = np.ascontiguousarray(_B[:, ::-1])  # (4, 4)

_CACHE = {}


def _register_op(name, spec):
    for o in dve_ops.OPS:
        if o.name == name:
            return o
    row = max(dve_ops._SUB_OPCODE_FOR_NAME.values()) + 1
    assert row < 0x20
    dve_ops._SUB_OPCODE_FOR_NAME[name] = row
    shas = {}
    for ver in ("v3", "v4"):
        u = lower(spec, ver=ver)
        shas[ver] = DveOpSpec(
            name=name, opcode=row, uops=u, rd1_en=_has_src1(spec)).sha(ver)
    op = dve_ops.DveOp(name, spec, subdim=False, uops_sha=shas)
    dve_ops.OPS.append(op)
    dve_ops.CUSTOM_DVE_SPECS[name] = spec
    return op


def _frac_ref(in0, in1, s0, s1, imm2):
    a = (np.asarray(in0, np.float32) * np.float32(s1)).astype(np.float32)
    r = ((a + np.float32(s0)).astype(np.float32) - np.float32(s0)).astype(np.float32)
    w = (a - r).astype(np.float32)
    return (w + (w < 0).astype(np.float32)).astype(np.float32)


def _acc2_ref(in0, in1, s0, s1, imm2):
    d = np.asarray(in0, np.float32)
    m0 = (d >= np.float32(imm2)).astype(np.float32)
    m1 = (d >= np.float32(imm2) + np.float32(1.0)).astype(np.float32)
    return (np.asarray(in1, np.float32) + m0 * np.asarray(s0, np.float32)
            + m1 * np.asarray(s1, np.float32)).astype(np.float32)


_a = Src0 * C1
_w = _a - ((_a + C0) - C0)
CR_FRAC = _register_op("CR_FRAC_ANT", Spec(
    body=_w + (_w < Zero),
    reference=_frac_ref))

CR_ACC2 = _register_op("CR_ACC2_ANT", Spec(
    body=Src1 + (Src0 >= C2) * C0 + (Src0 >= (C2 + One)) * C1,
    reference=_acc2_ref))


def _init2_ref(in0, in1, s0, s1, imm2):
    # in1 is the C3-spilled [P,1] scalar (second delta); s1 = base
    d = np.asarray(in0, np.float32)
    m0 = (d >= np.float32(imm2)).astype(np.float32)
    m1 = (d >= np.float32(imm2) + np.float32(1.0)).astype(np.float32)
    return (np.asarray(s1, np.float32) + m0 * np.asarray(s0, np.float32)
            + m1 * np.asarray(in1, np.float32)).astype(np.float32)


from concourse.dve_spec import C3, _spill_c3_to_src1  # noqa: E402

CR_INIT2 = _register_op("CR_INIT2_ANT", Spec(
    body=_spill_c3_to_src1(
        C1 + (Src0 >= C2) * C0 + (Src0 >= (C2 + One)) * C3),
    reference=_init2_ref))


def _build_bass(gens: int = 1):
    nc = bacc.Bacc("TRN2", target_bir_lowering=False, debug=False,
                   num_devices=NCORES)
    xr = nc.dram_tensor("xr", [NL, M], f32, kind="ExternalInput").ap()
    xct = nc.dram_tensor("xct", [NL, M], f32, kind="ExternalInput").ap()
    cp = nc.dram_tensor("cp", [NL, K], f32, kind="ExternalInput").ap()
    eye = nc.dram_tensor("eye", [P, P], bf16, kind="ExternalInput").ap()
    out = nc.dram_tensor("out", [NL, M], f32, kind="ExternalOutput").ap()

    NCH = FT // 512  # PSUM matmul chunks per tile

    with tile.TileContext(nc, num_cores=NCORES) as tc, ExitStack() as ctx:
        const_pool = ctx.enter_context(tc.tile_pool(name="const", bufs=1))
        in_pool = ctx.enter_context(tc.tile_pool(name="inp", bufs=2))
        du_pool = ctx.enter_context(tc.tile_pool(name="du", bufs=2))
        acc_pool = ctx.enter_context(tc.tile_pool(name="acc", bufs=1))
        hb_pool = ctx.enter_context(tc.tile_pool(name="hb", bufs=2))
        m_pool = ctx.enter_context(tc.tile_pool(name="m", bufs=2))
        out_pool = ctx.enter_context(tc.tile_pool(name="outp", bufs=2))
        ps_pool = ctx.enter_context(
            tc.tile_pool(name="ps", bufs=2, space="PSUM"))

        eye_t = const_pool.tile([P, P], bf16, tag="eye")
        nc.sync.dma_start(eye_t[:], eye[:, :])

        for jb in range(NL // P):  # two 128-neuron blocks
            # ---- tiny per-block table prep (on DVE; negligible) ----
            cpt = const_pool.tile([P, K], f32, tag=f"cp{jb}")
            nc.sync.dma_start(cpt[:], cp[jb * P:(jb + 1) * P, :])
            # T_t[:, vi] for vi=0..6 (v=vi+1); D_t[:, vi] = T(vi+1)-T(vi)
            T = [const_pool.tile([P, 7], f32, tag=f"T{jb}_{t}", name=f"T{jb}_{t}")
                 for t in range(4)]
            D = [const_pool.tile([P, 6], f32, tag=f"D{jb}_{t}", name=f"D{jb}_{t}")
                 for t in range(4)]
            for t in range(4):
                nc.vector.tensor_single_scalar(
                    T[t][:], cpt[:, 0:7], float(WREV[t, 0]), Alu.mult)
                for k in range(1, 4):
                    nc.vector.scalar_tensor_tensor(
                        T[t][:], cpt[:, k:k + 7], float(WREV[t, k]), T[t][:],
                        Alu.mult, Alu.add)
                nc.vector.tensor_sub(D[t][:], T[t][:, 1:7], T[t][:, 0:6])

            for fc in range(M // FT):
                fsl = slice(fc * FT, (fc + 1) * FT)
                psl = slice(jb * P, (jb + 1) * P)

                xr_t = in_pool.tile([P, FT], f32, tag="xr")
                nc.sync.dma_start(xr_t[:], xr[psl, fsl])
                xc_t = in_pool.tile([P, FT], f32, tag="xc")
                nc.sync.dma_start(xc_t[:], xct[psl, fsl])

                # segment line on ACT: d = 1.5*x + 4  (steps saturate, so
                # the reference's clamp at the range edges is automatic)
                d_t = du_pool.tile([P, FT], f32, tag="d")
                nc.scalar.activation(d_t[:], xc_t[:], Act.Copy,
                                     bias=4.0, scale=1.5)
                # u on DVE (exact round-to-nearest via magic constant)
                ub_t = du_pool.tile([P, FT], bf16, tag="ub")
                nc.vector._custom_dve(CR_FRAC, out=ub_t[:], in0=xr_t[:],
                                      s0=MAGIC, s1=2.0)
                # u^2 on ACT
                u2_t = du_pool.tile([P, FT], bf16, tag="u2")
                nc.scalar.activation(u2_t[:], ub_t[:], Act.Square)

                # H_t = T_t[:,1] + sum_{v=2..7} (d>=v) * D_t[:,v-2]
                # 3 custom insts per t, interleaved round-robin so
                # consecutive DVE insts avoid back-to-back RAW.
                hf = [acc_pool.tile([P, FT], f32, tag=f"h{t}", name=f"h{t}")
                      for t in range(4)]
                hb = [hb_pool.tile([P, FT], bf16, tag=f"hb{t}", name=f"hb{t}")
                      for t in range(4)]
                for t in range(4):
                    nc.vector._custom_dve(
                        CR_INIT2, out=hf[t][:], in0=d_t[:], in1=D[t][:, 1:2],
                        s0=D[t][:, 0:1], s1=T[t][:, 0:1], imm2=2.0)
                for t in range(4):
                    nc.vector._custom_dve(
                        CR_ACC2, out=hf[t][:], in0=d_t[:], in1=hf[t][:],
                        s0=D[t][:, 2:3], s1=D[t][:, 3:4], imm2=4.0)
                for t in range(4):
                    nc.vector._custom_dve(
                        CR_ACC2, out=hb[t][:], in0=d_t[:], in1=hf[t][:],
                        s0=D[t][:, 4:5], s1=D[t][:, 5:6], imm2=6.0)

                # powers-form products, bf16 2x-mode TT on DVE:
                #   m0 = (h0*u^2)*u, m1 = h1*u^2, m2 = h2*u
                t0 = m_pool.tile([P, FT], bf16, tag="t0")
                m0 = m_pool.tile([P, FT], bf16, tag="m0")
                m1 = m_pool.tile([P, FT], bf16, tag="m1")
                m2 = m_pool.tile([P, FT], bf16, tag="m2")
                nc.vector.tensor_tensor(t0[:], hb[0][:], u2_t[:], Alu.mult)
                nc.vector.tensor_tensor(m0[:], t0[:], ub_t[:], Alu.mult)
                nc.vector.tensor_tensor(m1[:], hb[1][:], u2_t[:], Alu.mult)
                nc.vector.tensor_tensor(m2[:], hb[2][:], ub_t[:], Alu.mult)

                # final sum on PE: psum = m0 + m1 + m2 + h3
                ps_t = ps_pool.tile([P, FT], f32, tag="ps")
                for c in range(NCH):
                    cs = slice(c * 512, (c + 1) * 512)
                    nc.tensor.matmul(ps_t[:, cs], eye_t[:], m0[:, cs],
                                     start=True, stop=False)
                    nc.tensor.matmul(ps_t[:, cs], eye_t[:], m1[:, cs],
                                     start=False, stop=False)
                    nc.tensor.matmul(ps_t[:, cs], eye_t[:], m2[:, cs],
                                     start=False, stop=False)
                    nc.tensor.matmul(ps_t[:, cs], eye_t[:], hb[3][:, cs],
                                     start=False, stop=True)

                # PSUM -> SBUF on ACT, then DMA out
                o_t = out_pool.tile([P, FT], f32, tag="o")
                nc.scalar.activation(o_t[:], ps_t[:], Act.Copy)
                nc.sync.dma_start(out[psl, fsl], o_t[:])

    nc.finalize()
    return nc


def _get_nc():
    if "nc" not in _CACHE:
        _CACHE["nc"] = _build_bass()
    return _CACHE["nc"]


def build_in_maps(x: np.ndarray, cp: np.ndarray) -> list[dict]:
    mrows = M // NCORES  # 1024 output rows per core
    eye = np.eye(P, dtype=ml_dtypes.bfloat16)
    in_maps = []
    for c in range(NCORES):
        xr = np.ascontiguousarray(
            x[c * mrows:(c + 1) * mrows, :]).reshape(NL, M)
        xct = np.ascontiguousarray(x[:, c * NL:(c + 1) * NL].T)
        cpc = np.ascontiguousarray(cp[c * NL:(c + 1) * NL, :])
        in_maps.append({"xr": xr, "xct": xct, "cp": cpc, "eye": eye})
    return in_maps


def kernel(x: np.ndarray, control_points: np.ndarray) -> np.ndarray:
    x = np.ascontiguousarray(np.asarray(x, dtype=np.float32))
    cp = np.ascontiguousarray(np.asarray(control_points, dtype=np.float32))
    assert x.shape == (M, N) and cp.shape == (N, K)

    nc = _get_nc()
    mrows = M // NCORES  # 1024 output rows per core
    in_maps = build_in_maps(x, cp)

    res = run_bass_kernel_spmd(nc, in_maps, core_ids=list(range(NCORES)))
    outs = [res.results[c]["out"].reshape(mrows, N) for c in range(NCORES)]
    return np.concatenate(outs, axis=0)


# revision 18
# speedup vs baseline: 1.4296x; 1.0522x over previous
"""Catmull-Rom activation kernel for 8 TRN2 NeuronCores.

Reference semantics (m=8192 samples, n=2048 neurons, K=10 control points):
  p0    = floor(((x+2)*6)/4 + 1), clamped to 1 at x<=-2 and 7 at x>=2
  u     = frac(2x)
  coef  = (U @ B)[:, ::-1]   with U = [u^3, u^2, u, 1]   (sample-major flat)
  Q_k   = CP[j, p0+k-1]                                  (neuron-major flat)
  out   = sum_k coef_k * Q_k elementwise ON MISMATCHED FLATTENS: at flat
          position p (sample-major), coef comes from x[p//n, p%n] while Q
          comes from neuron jq=p//m, sample iq=p%m.

Algebraic regrouping: out = H0*u^3 + H1*u^2 + H2*u + H3 where
  H_t = T_t[j, s],  T_t[j, v] = sum_k B[t, 3-k] * CP[j, v+k-1],  s in 1..7.

Per-core layout (core c of 8): all work happens in the neuron-major
"H layout" (256 neurons x 8192 samples). Each core receives
  xr  = x[1024c:1024(c+1), :].reshape(256, 8192)   (u side)
  xct = x[:, 256c:256(c+1)].T                       (segment side)
  cp  = control_points[256c:256(c+1), :]
and its (256, 8192) output block is exactly out rows [1024c, 1024(c+1))
reinterpreted. No collectives, no on-device transposes.

Engine assignment (GpSimd/Pool is deliberately IDLE: it shares an SBUF
port with the DVE and the two serialize, so any Pool op costs more port
time than the same op on DVE):
  Scalar/ACT (own SBUF port): d = 1.5*x+4 (segment line, steps saturate
      so no clamp needed); u^2 = Square(u); PSUM->SBUF output copy.
  Vector/DVE: u = frac(2x) via magic-rounding custom op (bf16 out);
      4 H_t step-accumulation chains (3 custom insts each, fp32,
      final inst writes bf16); 4 bf16 2x-mode tensor_tensor multiplies
      (h0*u^2, (h0*u^2)*u, h1*u^2, h2*u).
  Tensor/PE: final sum m0+m1+m2+h3 via identity-weight matmuls
      accumulated in PSUM (bf16 inputs, fp32 accumulate).
"""

import sys

import numpy as np

sys.path.insert(0, "/opt/trn_rl_repo")

from contextlib import ExitStack

import concourse.bass as bass
import concourse.bacc as bacc
import concourse.mybir as mybir
from concourse import tile
from concourse import dve_ops
from concourse.dve_spec import (
    Spec, Src0, Src1, C0, C1, C2, Zero, One, maxx, minn, lower, _has_src1,
)
from concourse.dve_uop import DveOpSpec
from concourse.bass_utils import run_bass_kernel_spmd

import ml_dtypes

M = 8192          # samples
N = 2048          # neurons
K = 10            # control points per neuron
NCORES = 8
NL = N // NCORES  # 256 neurons per core
P = 128           # partitions per tile
FT = 2048         # free-dim tile size
f32 = mybir.dt.float32
bf16 = mybir.dt.bfloat16
Alu = mybir.AluOpType
Act = mybir.ActivationFunctionType
MAGIC = 12582912.0  # 1.5 * 2^23: rne-to-integer bias, valid for |t| < 2^22

# Wrev[t, k] = B[t, 3-k]; T_t[:, v] = sum_k Wrev[t,k] * CP[:, v-1+k]
_B = 0.5 * np.array(
    [[-1.0, 3.0, -3.0, 1.0],
     [2.0, -5.0, 4.0, -1.0],
     [-1.0, 0.0, 1.0, 0.0],
     [0.0, 2.0, 0.0, 0.0]], dtype=np.float32)
WREV = np.ascontiguousarray(_B[:, ::-1])  # (4, 4)

_CACHE = {}


def _register_op(name, spec):
    for o in dve_ops.OPS:
        if o.name == name:
            return o
    row = max(dve_ops._SUB_OPCODE_FOR_NAME.values()) + 1
    assert row < 0x20
    dve_ops._SUB_OPCODE_FOR_NAME[name] = row
    shas = {}
    for ver in ("v3", "v4"):
        u = lower(spec, ver=ver)
        shas[ver] = DveOpSpec(
            name=name, opcode=row, uops=u, rd1_en=_has_src1(spec)).sha(ver)
    op = dve_ops.DveOp(name, spec, subdim=False, uops_sha=shas)
    dve_ops.OPS.append(op)
    dve_ops.CUSTOM_DVE_SPECS[name] = spec
    return op


def _frac_ref(in0, in1, s0, s1, imm2):
    a = (np.asarray(in0, np.float32) * np.float32(s1)).astype(np.float32)
    r = ((a + np.float32(s0)).astype(np.float32) - np.float32(s0)).astype(np.float32)
    w = (a - r).astype(np.float32)
    return (w + (w < 0).astype(np.float32)).astype(np.float32)


def _acc2_ref(in0, in1, s0, s1, imm2):
    d = np.asarray(in0, np.float32)
    m0 = (d >= np.float32(imm2)).astype(np.float32)
    m1 = (d >= np.float32(imm2) + np.float32(1.0)).astype(np.float32)
    return (np.asarray(in1, np.float32) + m0 * np.asarray(s0, np.float32)
            + m1 * np.asarray(s1, np.float32)).astype(np.float32)


_a = Src0 * C1
_w = _a - ((_a + C0) - C0)
CR_FRAC = _register_op("CR_FRAC_ANT", Spec(
    body=_w + (_w < Zero),
    reference=_frac_ref))

CR_ACC2 = _register_op("CR_ACC2_ANT", Spec(
    body=Src1 + (Src0 >= C2) * C0 + (Src0 >= (C2 + One)) * C1,
    reference=_acc2_ref))


def _init2_ref(in0, in1, s0, s1, imm2):
    # in1 is the C3-spilled [P,1] scalar (second delta); s1 = base
    d = np.asarray(in0, np.float32)
    m0 = (d >= np.float32(imm2)).astype(np.float32)
    m1 = (d >= np.float32(imm2) + np.float32(1.0)).astype(np.float32)
    return (np.asarray(s1, np.float32) + m0 * np.asarray(s0, np.float32)
            + m1 * np.asarray(in1, np.float32)).astype(np.float32)


from concourse.dve_spec import C3, _spill_c3_to_src1  # noqa: E402

CR_INIT2 = _register_op("CR_INIT2_ANT", Spec(
    body=_spill_c3_to_src1(
        C1 + (Src0 >= C2) * C0 + (Src0 >= (C2 + One)) * C3),
    reference=_init2_ref))


def _ss2_ref(in0, in1, s0, s1, imm2):
    # two step-scaled terms, single-src
    d = np.asarray(in0, np.float32)
    m0 = (d >= np.float32(imm2)).astype(np.float32)
    m1 = (d >= np.float32(imm2) + np.float32(1.0)).astype(np.float32)
    return (m0 * np.asarray(s0, np.float32)
            + m1 * np.asarray(s1, np.float32)).astype(np.float32)


CR_SS2 = _register_op("CR_SS2_ANT", Spec(
    body=(Src0 >= C2) * C0 + (Src0 >= (C2 + One)) * C1,
    reference=_ss2_ref))


def _build_bass(gens: int = 1):
    nc = bacc.Bacc("TRN2", target_bir_lowering=False, debug=False,
                   num_devices=NCORES)
    xr = nc.dram_tensor("xr", [NL, M], f32, kind="ExternalInput").ap()
    xct = nc.dram_tensor("xct", [NL, M], f32, kind="ExternalInput").ap()
    cp = nc.dram_tensor("cp", [NL, K], f32, kind="ExternalInput").ap()
    eye = nc.dram_tensor("eye", [P, P], bf16, kind="ExternalInput").ap()
    out = nc.dram_tensor("out", [NL, M], f32, kind="ExternalOutput").ap()

    NCH = FT // 512  # PSUM matmul chunks per tile

    with tile.TileContext(nc, num_cores=NCORES) as tc, ExitStack() as ctx:
        const_pool = ctx.enter_context(tc.tile_pool(name="const", bufs=1))
        in_pool = ctx.enter_context(tc.tile_pool(name="inp", bufs=2))
        du_pool = ctx.enter_context(tc.tile_pool(name="du", bufs=2))
        acc_pool = ctx.enter_context(tc.tile_pool(name="acc", bufs=1))
        hb_pool = ctx.enter_context(tc.tile_pool(name="hb", bufs=2))
        m_pool = ctx.enter_context(tc.tile_pool(name="m", bufs=1))
        sg_pool = ctx.enter_context(tc.tile_pool(name="sg", bufs=1))
        out_pool = ctx.enter_context(tc.tile_pool(name="outp", bufs=2))
        ps_pool = ctx.enter_context(
            tc.tile_pool(name="ps", bufs=2, space="PSUM"))

        eye_t = const_pool.tile([P, P], bf16, tag="eye")
        nc.sync.dma_start(eye_t[:], eye[:, :])

        for jb in range(NL // P):  # two 128-neuron blocks
            # ---- tiny per-block table prep (on DVE; negligible) ----
            cpt = const_pool.tile([P, K], f32, tag=f"cp{jb}")
            nc.sync.dma_start(cpt[:], cp[jb * P:(jb + 1) * P, :])
            # T_t[:, vi] for vi=0..6 (v=vi+1); D_t[:, vi] = T(vi+1)-T(vi)
            T = [const_pool.tile([P, 7], f32, tag=f"T{jb}_{t}", name=f"T{jb}_{t}")
                 for t in range(4)]
            D = [const_pool.tile([P, 6], f32, tag=f"D{jb}_{t}", name=f"D{jb}_{t}")
                 for t in range(4)]
            for t in range(4):
                nc.vector.tensor_single_scalar(
                    T[t][:], cpt[:, 0:7], float(WREV[t, 0]), Alu.mult)
                for k in range(1, 4):
                    nc.vector.scalar_tensor_tensor(
                        T[t][:], cpt[:, k:k + 7], float(WREV[t, k]), T[t][:],
                        Alu.mult, Alu.add)
                nc.vector.tensor_sub(D[t][:], T[t][:, 1:7], T[t][:, 0:6])
            # halves of the last two deltas of chain 3 (ACT sign-step path)
            d45h = const_pool.tile([P, 2], f32, tag=f"d45h{jb}",
                                   name=f"d45h{jb}")
            nc.vector.tensor_single_scalar(
                d45h[:], D[3][:, 4:6], 0.5, Alu.mult)
            # [P,1] bias constants -6/-7 for the ACT Sign steps
            cm67 = const_pool.tile([P, 2], f32, tag=f"cm67{jb}",
                                   name=f"cm67{jb}")
            nc.vector.memset(cm67[:, 0:1], -6.0)
            nc.vector.memset(cm67[:, 1:2], -7.0)

            for fc in range(M // FT):
                fsl = slice(fc * FT, (fc + 1) * FT)
                psl = slice(jb * P, (jb + 1) * P)

                xr_t = in_pool.tile([P, FT], f32, tag="xr")
                nc.sync.dma_start(xr_t[:], xr[psl, fsl])
                xc_t = in_pool.tile([P, FT], f32, tag="xc")
                nc.sync.dma_start(xc_t[:], xct[psl, fsl])

                # segment line on ACT: d = 1.5*x + 4  (steps saturate, so
                # the reference's clamp at the range edges is automatic)
                d_t = du_pool.tile([P, FT], f32, tag="d")
                nc.scalar.activation(d_t[:], xc_t[:], Act.Copy,
                                     bias=4.0, scale=1.5)
                # u on DVE (exact round-to-nearest via magic constant)
                ub_t = du_pool.tile([P, FT], bf16, tag="ub")
                nc.vector._custom_dve(CR_FRAC, out=ub_t[:], in0=xr_t[:],
                                      s0=MAGIC, s1=2.0)
                # u^2 on ACT
                u2_t = du_pool.tile([P, FT], bf16, tag="u2")
                nc.scalar.activation(u2_t[:], ub_t[:], Act.Square)

                # H_t = T_t[:,1] + sum_{v=2..7} (d>=v) * D_t[:,v-2]
                # t=0..2: 3 custom insts per t, interleaved round-robin so
                # consecutive DVE insts avoid back-to-back RAW.
                # t=3 enters the output additively, so it is never
                # materialized: two single-src customs (p13: base+v2+v3,
                # p23: v4+v5+v6) plus the v7 step on ACT (sign trick);
                # PE sums all three pieces straight into PSUM.
                hf = [acc_pool.tile([P, FT], f32, tag=f"h{t}", name=f"h{t}")
                      for t in range(3)]
                hb = [hb_pool.tile([P, FT], bf16, tag=f"hb{t}", name=f"hb{t}")
                      for t in range(3)]
                p13 = hb_pool.tile([P, FT], bf16, tag="p13")
                p23 = hb_pool.tile([P, FT], bf16, tag="p23")
                sg6 = sg_pool.tile([P, FT], bf16, tag="sg6")
                sg7 = sg_pool.tile([P, FT], bf16, tag="sg7")
                q6 = hb_pool.tile([P, FT], bf16, tag="q6")
                q7 = hb_pool.tile([P, FT], bf16, tag="q7")
                nc.vector._custom_dve(
                    CR_INIT2, out=p13[:], in0=d_t[:], in1=D[3][:, 1:2],
                    s0=D[3][:, 0:1], s1=T[3][:, 0:1], imm2=2.0)
                for t in range(3):
                    nc.vector._custom_dve(
                        CR_INIT2, out=hf[t][:], in0=d_t[:], in1=D[t][:, 1:2],
                        s0=D[t][:, 0:1], s1=T[t][:, 0:1], imm2=2.0)
                nc.vector._custom_dve(
                    CR_SS2, out=p23[:], in0=d_t[:],
                    s0=D[3][:, 2:3], s1=D[3][:, 3:4], imm2=4.0)
                nc.scalar.activation(sg6[:], d_t[:], Act.Sign,
                                     bias=cm67[:, 0:1])
                nc.scalar.activation(q6[:], sg6[:], Act.Identity,
                                     bias=d45h[:, 0:1], scale=d45h[:, 0:1])
                nc.scalar.activation(sg7[:], d_t[:], Act.Sign,
                                     bias=cm67[:, 1:2])
                nc.scalar.activation(q7[:], sg7[:], Act.Identity,
                                     bias=d45h[:, 1:2], scale=d45h[:, 1:2])
                for t in range(3):
                    nc.vector._custom_dve(
                        CR_ACC2, out=hf[t][:], in0=d_t[:], in1=hf[t][:],
                        s0=D[t][:, 2:3], s1=D[t][:, 3:4], imm2=4.0)
                for t in range(3):
                    nc.vector._custom_dve(
                        CR_ACC2, out=hb[t][:], in0=d_t[:], in1=hf[t][:],
                        s0=D[t][:, 4:5], s1=D[t][:, 5:6], imm2=6.0)

                # powers-form products, bf16 2x-mode TT on DVE:
                #   m0 = (h0*u^2)*u, m1 = h1*u^2, m2 = h2*u
                t0 = m_pool.tile([P, FT], bf16, tag="t0")
                m0 = m_pool.tile([P, FT], bf16, tag="m0")
                m1 = m_pool.tile([P, FT], bf16, tag="m1")
                m2 = m_pool.tile([P, FT], bf16, tag="m2")
                nc.vector.tensor_tensor(t0[:], hb[0][:], u2_t[:], Alu.mult)
                nc.vector.tensor_tensor(m0[:], t0[:], ub_t[:], Alu.mult)
                nc.vector.tensor_tensor(m1[:], hb[1][:], u2_t[:], Alu.mult)
                nc.vector.tensor_tensor(m2[:], hb[2][:], ub_t[:], Alu.mult)

                # final sum on PE: psum = m0 + m1 + m2 + (p13 + p23 + q3)
                ps_t = ps_pool.tile([P, FT], f32, tag="ps")
                for c in range(NCH):
                    cs = slice(c * 512, (c + 1) * 512)
                    nc.tensor.matmul(ps_t[:, cs], eye_t[:], m0[:, cs],
                                     start=True, stop=False)
                    nc.tensor.matmul(ps_t[:, cs], eye_t[:], m1[:, cs],
                                     start=False, stop=False)
                    nc.tensor.matmul(ps_t[:, cs], eye_t[:], m2[:, cs],
                                     start=False, stop=False)
                    nc.tensor.matmul(ps_t[:, cs], eye_t[:], p13[:, cs],
                                     start=False, stop=False)
                    nc.tensor.matmul(ps_t[:, cs], eye_t[:], p23[:, cs],
                                     start=False, stop=False)
                    nc.tensor.matmul(ps_t[:, cs], eye_t[:], q6[:, cs],
                                     start=False, stop=False)
                    nc.tensor.matmul(ps_t[:, cs], eye_t[:], q7[:, cs],
                                     start=False, stop=True)

                # PSUM -> SBUF on ACT, then DMA out
                o_t = out_pool.tile([P, FT], f32, tag="o")
                nc.scalar.activation(o_t[:], ps_t[:], Act.Copy)
                nc.sync.dma_start(out[psl, fsl], o_t[:])

    nc.finalize()
    return nc


def _get_nc():
    if "nc" not in _CACHE:
        _CACHE["nc"] = _build_bass()
    return _CACHE["nc"]


def build_in_maps(x: np.ndarray, cp: np.ndarray) -> list[dict]:
    mrows = M // NCORES  # 1024 output rows per core
    eye = np.eye(P, dtype=ml_dtypes.bfloat16)
    in_maps = []
    for c in range(NCORES):
        xr = np.ascontiguousarray(
            x[c * mrows:(c + 1) * mrows, :]).reshape(NL, M)
        xct = np.ascontiguousarray(x[:, c * NL:(c + 1) * NL].T)
        cpc = np.ascontiguousarray(cp[c * NL:(c + 1) * NL, :])
        in_maps.append({"xr": xr, "xct": xct, "cp": cpc, "eye": eye})
    return in_maps


def kernel(x: np.ndarray, control_points: np.ndarray) -> np.ndarray:
    x = np.ascontiguousarray(np.asarray(x, dtype=np.float32))
    cp = np.ascontiguousarray(np.asarray(control_points, dtype=np.float32))
    assert x.shape == (M, N) and cp.shape == (N, K)

    nc = _get_nc()
    mrows = M // NCORES  # 1024 output rows per core
    in_maps = build_in_maps(x, cp)

    res = run_bass_kernel_spmd(nc, in_maps, core_ids=list(range(NCORES)))
    outs = [res.results[c]["out"].reshape(mrows, N) for c in range(NCORES)]
    return np.concatenate(outs, axis=0)


# revision 21
# speedup vs baseline: 1.5128x; 1.0582x over previous
"""Catmull-Rom activation kernel for 8 TRN2 NeuronCores.

Reference semantics (m=8192 samples, n=2048 neurons, K=10 control points):
  p0    = floor(((x+2)*6)/4 + 1), clamped to 1 at x<=-2 and 7 at x>=2
  u     = frac(2x)
  coef  = (U @ B)[:, ::-1]   with U = [u^3, u^2, u, 1]   (sample-major flat)
  Q_k   = CP[j, p0+k-1]                                  (neuron-major flat)
  out   = sum_k coef_k * Q_k elementwise ON MISMATCHED FLATTENS: at flat
          position p (sample-major), coef comes from x[p//n, p%n] while Q
          comes from neuron jq=p//m, sample iq=p%m.

Algebraic regrouping: out = H0*u^3 + H1*u^2 + H2*u + H3 where
  H_t = T_t[j, s],  T_t[j, v] = sum_k B[t, 3-k] * CP[j, v+k-1],  s in 1..7.

Per-core layout (core c of 8): all work happens in the neuron-major
"H layout" (256 neurons x 8192 samples). Each core receives
  xr  = x[1024c:1024(c+1), :].reshape(256, 8192)   (u side)
  xct = x[:, 256c:256(c+1)].T                       (segment side)
  cp  = control_points[256c:256(c+1), :]
and its (256, 8192) output block is exactly out rows [1024c, 1024(c+1))
reinterpreted. No collectives, no on-device transposes.

Engine assignment (GpSimd/Pool is deliberately IDLE: it shares an SBUF
port with the DVE and the two serialize, so any Pool op costs more port
time than the same op on DVE):
  Scalar/ACT (own SBUF port): d = 1.5*x+4 (segment line, steps saturate
      so no clamp needed); u^2 = Square(u); PSUM->SBUF output copy.
  Vector/DVE: u = frac(2x) via magic-rounding custom op (bf16 out);
      4 H_t step-accumulation chains (3 custom insts each, fp32,
      final inst writes bf16); 4 bf16 2x-mode tensor_tensor multiplies
      (h0*u^2, (h0*u^2)*u, h1*u^2, h2*u).
  Tensor/PE: final sum m0+m1+m2+h3 via identity-weight matmuls
      accumulated in PSUM (bf16 inputs, fp32 accumulate).
"""

import sys

import numpy as np

sys.path.insert(0, "/opt/trn_rl_repo")

from contextlib import ExitStack

import concourse.bass as bass
import concourse.bacc as bacc
import concourse.mybir as mybir
from concourse import tile
from concourse import dve_ops
from concourse.dve_spec import (
    Spec, Src0, Src1, C0, C1, C2, Zero, One, maxx, minn, lower, _has_src1,
)
from concourse.dve_uop import DveOpSpec
from concourse.bass_utils import run_bass_kernel_spmd

import ml_dtypes

M = 8192          # samples
N = 2048          # neurons
K = 10            # control points per neuron
NCORES = 8
NL = N // NCORES  # 256 neurons per core
P = 128           # partitions per tile
FT = 2048         # free-dim tile size
f32 = mybir.dt.float32
bf16 = mybir.dt.bfloat16
Alu = mybir.AluOpType
Act = mybir.ActivationFunctionType
MAGIC = 12582912.0  # 1.5 * 2^23: rne-to-integer bias, valid for |t| < 2^22

# Wrev[t, k] = B[t, 3-k]; T_t[:, v] = sum_k Wrev[t,k] * CP[:, v-1+k]
_B = 0.5 * np.array(
    [[-1.0, 3.0, -3.0, 1.0],
     [2.0, -5.0, 4.0, -1.0],
     [-1.0, 0.0, 1.0, 0.0],
     [0.0, 2.0, 0.0, 0.0]], dtype=np.float32)
WREV = np.ascontiguousarray(_B[:, ::-1])  # (4, 4)

_CACHE = {}


def _register_op(name, spec):
    for o in dve_ops.OPS:
        if o.name == name:
            return o
    row = max(dve_ops._SUB_OPCODE_FOR_NAME.values()) + 1
    assert row < 0x20
    dve_ops._SUB_OPCODE_FOR_NAME[name] = row
    shas = {}
    for ver in ("v3", "v4"):
        u = lower(spec, ver=ver)
        shas[ver] = DveOpSpec(
            name=name, opcode=row, uops=u, rd1_en=_has_src1(spec)).sha(ver)
    op = dve_ops.DveOp(name, spec, subdim=False, uops_sha=shas)
    dve_ops.OPS.append(op)
    dve_ops.CUSTOM_DVE_SPECS[name] = spec
    return op


def _frac_ref(in0, in1, s0, s1, imm2):
    a = (np.asarray(in0, np.float32) * np.float32(s1)).astype(np.float32)
    r = ((a + np.float32(s0)).astype(np.float32) - np.float32(s0)).astype(np.float32)
    w = (a - r).astype(np.float32)
    return (w + (w < 0).astype(np.float32)).astype(np.float32)


def _acc2_ref(in0, in1, s0, s1, imm2):
    d = np.asarray(in0, np.float32)
    m0 = (d >= np.float32(imm2)).astype(np.float32)
    m1 = (d >= np.float32(imm2) + np.float32(1.0)).astype(np.float32)
    return (np.asarray(in1, np.float32) + m0 * np.asarray(s0, np.float32)
            + m1 * np.asarray(s1, np.float32)).astype(np.float32)


_a = Src0 * C1
_w = _a - ((_a + C0) - C0)
CR_FRAC = _register_op("CR_FRAC_ANT", Spec(
    body=_w + (_w < Zero),
    reference=_frac_ref))

CR_ACC2 = _register_op("CR_ACC2_ANT", Spec(
    body=Src1 + (Src0 >= C2) * C0 + (Src0 >= (C2 + One)) * C1,
    reference=_acc2_ref))


def _init2_ref(in0, in1, s0, s1, imm2):
    # in1 is the C3-spilled [P,1] scalar (second delta); s1 = base
    d = np.asarray(in0, np.float32)
    m0 = (d >= np.float32(imm2)).astype(np.float32)
    m1 = (d >= np.float32(imm2) + np.float32(1.0)).astype(np.float32)
    return (np.asarray(s1, np.float32) + m0 * np.asarray(s0, np.float32)
            + m1 * np.asarray(in1, np.float32)).astype(np.float32)


from concourse.dve_spec import C3, _spill_c3_to_src1  # noqa: E402

CR_INIT2 = _register_op("CR_INIT2_ANT", Spec(
    body=_spill_c3_to_src1(
        C1 + (Src0 >= C2) * C0 + (Src0 >= (C2 + One)) * C3),
    reference=_init2_ref))


def _ss2_ref(in0, in1, s0, s1, imm2):
    # two step-scaled terms, single-src
    d = np.asarray(in0, np.float32)
    m0 = (d >= np.float32(imm2)).astype(np.float32)
    m1 = (d >= np.float32(imm2) + np.float32(1.0)).astype(np.float32)
    return (m0 * np.asarray(s0, np.float32)
            + m1 * np.asarray(s1, np.float32)).astype(np.float32)


CR_SS2 = _register_op("CR_SS2_ANT", Spec(
    body=(Src0 >= C2) * C0 + (Src0 >= (C2 + One)) * C1,
    reference=_ss2_ref))


def _build_bass(gens: int = 1):
    nc = bacc.Bacc("TRN2", target_bir_lowering=False, debug=False,
                   num_devices=NCORES)
    xr = nc.dram_tensor("xr", [NL, M], f32, kind="ExternalInput").ap()
    xct = nc.dram_tensor("xct", [NL, M], f32, kind="ExternalInput").ap()
    cp = nc.dram_tensor("cp", [NL, K], f32, kind="ExternalInput").ap()
    eye = nc.dram_tensor("eye", [P, P], bf16, kind="ExternalInput").ap()
    out = nc.dram_tensor("out", [NL, M], f32, kind="ExternalOutput").ap()

    NCH = FT // 512  # PSUM matmul chunks per tile

    with tile.TileContext(nc, num_cores=NCORES) as tc, ExitStack() as ctx:
        const_pool = ctx.enter_context(tc.tile_pool(name="const", bufs=1))
        in_pool = ctx.enter_context(tc.tile_pool(name="inp", bufs=2))
        du_pool = ctx.enter_context(tc.tile_pool(name="du", bufs=2))
        acc_pool = ctx.enter_context(tc.tile_pool(name="acc", bufs=1))
        hb_pool = ctx.enter_context(tc.tile_pool(name="hb", bufs=2))
        m_pool = ctx.enter_context(tc.tile_pool(name="m", bufs=1))
        sg_pool = ctx.enter_context(tc.tile_pool(name="sg", bufs=1))
        out_pool = ctx.enter_context(tc.tile_pool(name="outp", bufs=2))
        ps_pool = ctx.enter_context(
            tc.tile_pool(name="ps", bufs=2, space="PSUM"))

        eye_t = const_pool.tile([P, P], bf16, tag="eye")
        nc.sync.dma_start(eye_t[:], eye[:, :])

        for jb in range(NL // P):  # two 128-neuron blocks
            # ---- tiny per-block table prep (on DVE; negligible) ----
            cpt = const_pool.tile([P, K], f32, tag=f"cp{jb}")
            nc.sync.dma_start(cpt[:], cp[jb * P:(jb + 1) * P, :])
            # T_t[:, vi] for vi=0..6 (v=vi+1); D_t[:, vi] = T(vi+1)-T(vi)
            T = [const_pool.tile([P, 7], f32, tag=f"T{jb}_{t}", name=f"T{jb}_{t}")
                 for t in range(4)]
            D = [const_pool.tile([P, 6], f32, tag=f"D{jb}_{t}", name=f"D{jb}_{t}")
                 for t in range(4)]
            for t in range(4):
                nc.vector.tensor_single_scalar(
                    T[t][:], cpt[:, 0:7], float(WREV[t, 0]), Alu.mult)
                for k in range(1, 4):
                    nc.vector.scalar_tensor_tensor(
                        T[t][:], cpt[:, k:k + 7], float(WREV[t, k]), T[t][:],
                        Alu.mult, Alu.add)
                nc.vector.tensor_sub(D[t][:], T[t][:, 1:7], T[t][:, 0:6])
            # chain 3 runs entirely on ACT as sign-steps: halves of all six
            # deltas, the base folded into the first step's bias, and the
            # [P,1] threshold constants -2..-7
            dh3 = const_pool.tile([P, 6], f32, tag=f"dh3{jb}",
                                  name=f"dh3{jb}")
            nc.vector.tensor_single_scalar(
                dh3[:], D[3][:, 0:6], 0.5, Alu.mult)
            b3p = const_pool.tile([P, 1], f32, tag=f"b3p{jb}",
                                  name=f"b3p{jb}")
            nc.vector.tensor_add(b3p[:], T[3][:, 0:1], dh3[:, 0:1])
            cmv = const_pool.tile([P, 6], f32, tag=f"cmv{jb}",
                                  name=f"cmv{jb}")
            for v in range(6):
                nc.vector.memset(cmv[:, v:v + 1], -2.0 - v)

            for fc in range(M // FT):
                fsl = slice(fc * FT, (fc + 1) * FT)
                psl = slice(jb * P, (jb + 1) * P)

                xr_t = in_pool.tile([P, FT], f32, tag="xr")
                nc.sync.dma_start(xr_t[:], xr[psl, fsl])
                xc_t = in_pool.tile([P, FT], f32, tag="xc")
                nc.sync.dma_start(xc_t[:], xct[psl, fsl])

                # segment line on ACT: d = 1.5*x + 4  (steps saturate, so
                # the reference's clamp at the range edges is automatic)
                d_t = du_pool.tile([P, FT], f32, tag="d")
                nc.scalar.activation(d_t[:], xc_t[:], Act.Copy,
                                     bias=4.0, scale=1.5)
                # u on DVE (exact round-to-nearest via magic constant)
                ub_t = du_pool.tile([P, FT], bf16, tag="ub")
                nc.vector._custom_dve(CR_FRAC, out=ub_t[:], in0=xr_t[:],
                                      s0=MAGIC, s1=2.0)
                # u^2 on ACT
                u2_t = du_pool.tile([P, FT], bf16, tag="u2")
                nc.scalar.activation(u2_t[:], ub_t[:], Act.Square)

                # H_t = T_t[:,1] + sum_{v=2..7} (d>=v) * D_t[:,v-2]
                # t=0..2: 3 custom insts per t, interleaved round-robin so
                # consecutive DVE insts avoid back-to-back RAW.
                # t=3 enters the output additively, so it is never
                # materialized: each of its six steps is an ACT sign-step
                #   c_v = (D_v/2)*sign(d-v) + (D_v/2)   [base folded in c_2]
                # and PE sums the six pieces straight into PSUM.
                hf = [acc_pool.tile([P, FT], f32, tag=f"h{t}", name=f"h{t}")
                      for t in range(3)]
                hb = [hb_pool.tile([P, FT], bf16, tag=f"hb{t}", name=f"hb{t}")
                      for t in range(3)]
                cq = [hb_pool.tile([P, FT], bf16, tag=f"cq{v}", name=f"cq{v}")
                      for v in range(6)]
                for t in range(3):
                    nc.vector._custom_dve(
                        CR_INIT2, out=hf[t][:], in0=d_t[:], in1=D[t][:, 1:2],
                        s0=D[t][:, 0:1], s1=T[t][:, 0:1], imm2=2.0)
                for v in range(6):
                    sg = sg_pool.tile([P, FT], bf16, tag="sg")
                    nc.scalar.activation(sg[:], d_t[:], Act.Sign,
                                         bias=cmv[:, v:v + 1])
                    bias_ap = b3p[:, 0:1] if v == 0 else dh3[:, v:v + 1]
                    nc.scalar.activation(cq[v][:], sg[:], Act.Identity,
                                         bias=bias_ap, scale=dh3[:, v:v + 1])
                for t in range(3):
                    nc.vector._custom_dve(
                        CR_ACC2, out=hf[t][:], in0=d_t[:], in1=hf[t][:],
                        s0=D[t][:, 2:3], s1=D[t][:, 3:4], imm2=4.0)
                for t in range(3):
                    nc.vector._custom_dve(
                        CR_ACC2, out=hb[t][:], in0=d_t[:], in1=hf[t][:],
                        s0=D[t][:, 4:5], s1=D[t][:, 5:6], imm2=6.0)

                # powers-form products, bf16 2x-mode TT on DVE:
                #   m0 = (h0*u^2)*u, m1 = h1*u^2, m2 = h2*u
                t0 = m_pool.tile([P, FT], bf16, tag="t0")
                m0 = m_pool.tile([P, FT], bf16, tag="m0")
                m1 = m_pool.tile([P, FT], bf16, tag="m1")
                m2 = m_pool.tile([P, FT], bf16, tag="m2")
                nc.vector.tensor_tensor(t0[:], hb[0][:], u2_t[:], Alu.mult)
                nc.vector.tensor_tensor(m0[:], t0[:], ub_t[:], Alu.mult)
                nc.vector.tensor_tensor(m1[:], hb[1][:], u2_t[:], Alu.mult)
                nc.vector.tensor_tensor(m2[:], hb[2][:], ub_t[:], Alu.mult)

                # final sum on PE: psum = m0 + m1 + m2 + sum_v c_v
                ps_t = ps_pool.tile([P, FT], f32, tag="ps")
                for c in range(NCH):
                    cs = slice(c * 512, (c + 1) * 512)
                    nc.tensor.matmul(ps_t[:, cs], eye_t[:], m0[:, cs],
                                     start=True, stop=False)
                    nc.tensor.matmul(ps_t[:, cs], eye_t[:], m1[:, cs],
                                     start=False, stop=False)
                    nc.tensor.matmul(ps_t[:, cs], eye_t[:], m2[:, cs],
                                     start=False, stop=False)
                    for v in range(6):
                        nc.tensor.matmul(ps_t[:, cs], eye_t[:], cq[v][:, cs],
                                         start=False, stop=(v == 5))

                # PSUM -> SBUF on ACT, then DMA out
                o_t = out_pool.tile([P, FT], f32, tag="o")
                nc.scalar.activation(o_t[:], ps_t[:], Act.Copy)
                nc.sync.dma_start(out[psl, fsl], o_t[:])

    nc.finalize()
    return nc


def _get_nc():
    if "nc" not in _CACHE:
        _CACHE["nc"] = _build_bass()
    return _CACHE["nc"]


def build_in_maps(x: np.ndarray, cp: np.ndarray) -> list[dict]:
    mrows = M // NCORES  # 1024 output rows per core
    eye = np.eye(P, dtype=ml_dtypes.bfloat16)
    in_maps = []
    for c in range(NCORES):
        xr = np.ascontiguousarray(
            x[c * mrows:(c + 1) * mrows, :]).reshape(NL, M)
        xct = np.ascontiguousarray(x[:, c * NL:(c + 1) * NL].T)
        cpc = np.ascontiguousarray(cp[c * NL:(c + 1) * NL, :])
        in_maps.append({"xr": xr, "xct": xct, "cp": cpc, "eye": eye})
    return in_maps


def kernel(x: np.ndarray, control_points: np.ndarray) -> np.ndarray:
    x = np.ascontiguousarray(np.asarray(x, dtype=np.float32))
    cp = np.ascontiguousarray(np.asarray(control_points, dtype=np.float32))
    assert x.shape == (M, N) and cp.shape == (N, K)

    nc = _get_nc()
    mrows = M // NCORES  # 1024 output rows per core
    in_maps = build_in_maps(x, cp)

    res = run_bass_kernel_spmd(nc, in_maps, core_ids=list(range(NCORES)))
    outs = [res.results[c]["out"].reshape(mrows, N) for c in range(NCORES)]
    return np.concatenate(outs, axis=0)


# revision 26
# speedup vs baseline: 1.6085x; 1.0632x over previous
"""Catmull-Rom activation kernel for 8 TRN2 NeuronCores.

Reference semantics (m=8192 samples, n=2048 neurons, K=10 control points):
  p0    = floor(((x+2)*6)/4 + 1), clamped to 1 at x<=-2 and 7 at x>=2
  u     = frac(2x)
  coef  = (U @ B)[:, ::-1]   with U = [u^3, u^2, u, 1]   (sample-major flat)
  Q_k   = CP[j, p0+k-1]                                  (neuron-major flat)
  out   = sum_k coef_k * Q_k elementwise ON MISMATCHED FLATTENS: at flat
          position p (sample-major), coef comes from x[p//n, p%n] while Q
          comes from neuron jq=p//m, sample iq=p%m.

Algebraic regrouping: out = H0*u^3 + H1*u^2 + H2*u + H3 where
  H_t = T_t[j, s],  T_t[j, v] = sum_k B[t, 3-k] * CP[j, v+k-1],  s in 1..7.

Per-core layout (core c of 8): all work happens in the neuron-major
"H layout" (256 neurons x 8192 samples). Each core receives
  xr  = x[1024c:1024(c+1), :].reshape(256, 8192)   (u side)
  xct = x[:, 256c:256(c+1)].T                       (segment side)
  cp  = control_points[256c:256(c+1), :]
and its (256, 8192) output block is exactly out rows [1024c, 1024(c+1))
reinterpreted. No collectives, no on-device transposes.

Engine assignment (GpSimd/Pool is deliberately IDLE: it shares an SBUF
port with the DVE and the two serialize, so any Pool op costs more port
time than the same op on DVE):
  Scalar/ACT (own SBUF port): d = 1.5*x+4 (segment line, steps saturate
      so no clamp needed); u^2 = Square(u); PSUM->SBUF output copy.
  Vector/DVE: u = frac(2x) via magic-rounding custom op (bf16 out);
      4 H_t step-accumulation chains (3 custom insts each, fp32,
      final inst writes bf16); 4 bf16 2x-mode tensor_tensor multiplies
      (h0*u^2, (h0*u^2)*u, h1*u^2, h2*u).
  Tensor/PE: final sum m0+m1+m2+h3 via identity-weight matmuls
      accumulated in PSUM (bf16 inputs, fp32 accumulate).
"""

import sys

import numpy as np

sys.path.insert(0, "/opt/trn_rl_repo")

from contextlib import ExitStack

import concourse.bass as bass
import concourse.bacc as bacc
import concourse.mybir as mybir
from concourse import tile
from concourse import dve_ops
from concourse.dve_spec import (
    Spec, Src0, Src1, C0, C1, C2, Zero, One, maxx, minn, lower, _has_src1,
)
from concourse.dve_uop import DveOpSpec
from concourse.bass_utils import run_bass_kernel_spmd

import ml_dtypes

M = 8192          # samples
N = 2048          # neurons
K = 10            # control points per neuron
NCORES = 8
NL = N // NCORES  # 256 neurons per core
P = 128           # partitions per tile
FT = 2048         # free-dim tile size
f32 = mybir.dt.float32
bf16 = mybir.dt.bfloat16
Alu = mybir.AluOpType
Act = mybir.ActivationFunctionType
MAGIC = 12582912.0  # 1.5 * 2^23: rne-to-integer bias, valid for |t| < 2^22

# Wrev[t, k] = B[t, 3-k]; T_t[:, v] = sum_k Wrev[t,k] * CP[:, v-1+k]
_B = 0.5 * np.array(
    [[-1.0, 3.0, -3.0, 1.0],
     [2.0, -5.0, 4.0, -1.0],
     [-1.0, 0.0, 1.0, 0.0],
     [0.0, 2.0, 0.0, 0.0]], dtype=np.float32)
WREV = np.ascontiguousarray(_B[:, ::-1])  # (4, 4)

_CACHE = {}


def _register_op(name, spec):
    for o in dve_ops.OPS:
        if o.name == name:
            return o
    row = max(dve_ops._SUB_OPCODE_FOR_NAME.values()) + 1
    assert row < 0x20
    dve_ops._SUB_OPCODE_FOR_NAME[name] = row
    shas = {}
    for ver in ("v3", "v4"):
        u = lower(spec, ver=ver)
        shas[ver] = DveOpSpec(
            name=name, opcode=row, uops=u, rd1_en=_has_src1(spec)).sha(ver)
    op = dve_ops.DveOp(name, spec, subdim=False, uops_sha=shas)
    dve_ops.OPS.append(op)
    dve_ops.CUSTOM_DVE_SPECS[name] = spec
    return op


def _frac_ref(in0, in1, s0, s1, imm2):
    a = (np.asarray(in0, np.float32) * np.float32(s1)).astype(np.float32)
    r = ((a + np.float32(s0)).astype(np.float32) - np.float32(s0)).astype(np.float32)
    w = (a - r).astype(np.float32)
    return (w + (w < 0).astype(np.float32)).astype(np.float32)


def _acc2_ref(in0, in1, s0, s1, imm2):
    d = np.asarray(in0, np.float32)
    m0 = (d >= np.float32(imm2)).astype(np.float32)
    m1 = (d >= np.float32(imm2) + np.float32(1.0)).astype(np.float32)
    return (np.asarray(in1, np.float32) + m0 * np.asarray(s0, np.float32)
            + m1 * np.asarray(s1, np.float32)).astype(np.float32)


_a = Src0 * C1
_w = _a - ((_a + C0) - C0)
CR_FRAC = _register_op("CR_FRAC_ANT", Spec(
    body=_w + (_w < Zero),
    reference=_frac_ref))

CR_ACC2 = _register_op("CR_ACC2_ANT", Spec(
    body=Src1 + (Src0 >= C2) * C0 + (Src0 >= (C2 + One)) * C1,
    reference=_acc2_ref))


def _init2_ref(in0, in1, s0, s1, imm2):
    # in1 is the C3-spilled [P,1] scalar (second delta); s1 = base
    d = np.asarray(in0, np.float32)
    m0 = (d >= np.float32(imm2)).astype(np.float32)
    m1 = (d >= np.float32(imm2) + np.float32(1.0)).astype(np.float32)
    return (np.asarray(s1, np.float32) + m0 * np.asarray(s0, np.float32)
            + m1 * np.asarray(in1, np.float32)).astype(np.float32)


from concourse.dve_spec import C3, _spill_c3_to_src1  # noqa: E402

CR_INIT2 = _register_op("CR_INIT2_ANT", Spec(
    body=_spill_c3_to_src1(
        C1 + (Src0 >= C2) * C0 + (Src0 >= (C2 + One)) * C3),
    reference=_init2_ref))


def _ss2_ref(in0, in1, s0, s1, imm2):
    # two step-scaled terms, single-src
    d = np.asarray(in0, np.float32)
    m0 = (d >= np.float32(imm2)).astype(np.float32)
    m1 = (d >= np.float32(imm2) + np.float32(1.0)).astype(np.float32)
    return (m0 * np.asarray(s0, np.float32)
            + m1 * np.asarray(s1, np.float32)).astype(np.float32)


CR_SS2 = _register_op("CR_SS2_ANT", Spec(
    body=(Src0 >= C2) * C0 + (Src0 >= (C2 + One)) * C1,
    reference=_ss2_ref))


def _build_bass(gens: int = 1):
    nc = bacc.Bacc("TRN2", target_bir_lowering=False, debug=False,
                   num_devices=NCORES)
    xr = nc.dram_tensor("xr", [NL, M], f32, kind="ExternalInput").ap()
    xct = nc.dram_tensor("xct", [NL, M], f32, kind="ExternalInput").ap()
    cp = nc.dram_tensor("cp", [NL, K], f32, kind="ExternalInput").ap()
    eye = nc.dram_tensor("eye", [P, P], bf16, kind="ExternalInput").ap()
    dgt = nc.dram_tensor("dg3", [2 * 6, P, P], bf16, kind="ExternalInput").ap()
    out = nc.dram_tensor("out", [NL, M], f32, kind="ExternalOutput").ap()

    NCH = FT // 512  # PSUM matmul chunks per tile

    with tile.TileContext(nc, num_cores=NCORES) as tc, ExitStack() as ctx:
        const_pool = ctx.enter_context(tc.tile_pool(name="const", bufs=1))
        in_pool = ctx.enter_context(tc.tile_pool(name="inp", bufs=2))
        du_pool = ctx.enter_context(tc.tile_pool(name="du", bufs=2))
        acc_pool = ctx.enter_context(tc.tile_pool(name="acc", bufs=1))
        hb_pool = ctx.enter_context(tc.tile_pool(name="hb", bufs=2))
        m_pool = ctx.enter_context(tc.tile_pool(name="m", bufs=1))
        sg_pool = ctx.enter_context(tc.tile_pool(name="sg", bufs=1))
        out_pool = ctx.enter_context(tc.tile_pool(name="outp", bufs=2))
        ps_pool = ctx.enter_context(
            tc.tile_pool(name="ps", bufs=2, space="PSUM"))

        eye_t = const_pool.tile([P, P], bf16, tag="eye")
        nc.sync.dma_start(eye_t[:], eye[:, :])

        for jb in range(NL // P):  # two 128-neuron blocks
            # ---- tiny per-block table prep (on DVE; negligible) ----
            cpt = const_pool.tile([P, K], f32, tag=f"cp{jb}")
            nc.sync.dma_start(cpt[:], cp[jb * P:(jb + 1) * P, :])
            # T_t[:, vi] for vi=0..6 (v=vi+1); D_t[:, vi] = T(vi+1)-T(vi)
            T = [const_pool.tile([P, 7], f32, tag=f"T{jb}_{t}", name=f"T{jb}_{t}")
                 for t in range(4)]
            D = [const_pool.tile([P, 6], f32, tag=f"D{jb}_{t}", name=f"D{jb}_{t}")
                 for t in range(4)]
            for t in range(4):
                nc.vector.tensor_single_scalar(
                    T[t][:], cpt[:, 0:7], float(WREV[t, 0]), Alu.mult)
                for k in range(1, 4):
                    nc.vector.scalar_tensor_tensor(
                        T[t][:], cpt[:, k:k + 7], float(WREV[t, k]), T[t][:],
                        Alu.mult, Alu.add)
                nc.vector.tensor_sub(D[t][:], T[t][:, 1:7], T[t][:, 0:6])
            # chain 3 runs as ACT sign-steps + PE diagonal-weight scaling:
            # psum += diag(D3_v/2) @ sign(d-v); all the +D/2 offsets and the
            # base collapse to bias3 = 0.5*(cp[:,2]+cp[:,8]) applied in the
            # final PSUM->SBUF copy. diag weights are shipped from the host.
            b3t = const_pool.tile([P, 1], f32, tag=f"b3t{jb}",
                                  name=f"b3t{jb}")
            nc.vector.tensor_add(b3t[:], cpt[:, 2:3], cpt[:, 8:9])
            nc.vector.tensor_single_scalar(b3t[:], b3t[:], 0.5, Alu.mult)
            dg3 = [const_pool.tile([P, P], bf16, tag=f"dg{jb}_{v}",
                                   name=f"dg{jb}_{v}") for v in range(6)]
            for v in range(6):
                nc.sync.dma_start(dg3[v][:], dgt[jb * 6 + v, :, :])
            cmv = const_pool.tile([P, 6], f32, tag=f"cmv{jb}",
                                  name=f"cmv{jb}")
            for v in range(6):
                nc.vector.memset(cmv[:, v:v + 1], -2.0 - v)

            for fc in range(M // FT):
                fsl = slice(fc * FT, (fc + 1) * FT)
                psl = slice(jb * P, (jb + 1) * P)

                xr_t = in_pool.tile([P, FT], f32, tag="xr")
                nc.sync.dma_start(xr_t[:], xr[psl, fsl])
                xc_t = in_pool.tile([P, FT], f32, tag="xc")
                nc.sync.dma_start(xc_t[:], xct[psl, fsl])

                # segment line on ACT: d = 1.5*x + 4  (steps saturate, so
                # the reference's clamp at the range edges is automatic)
                d_t = du_pool.tile([P, FT], f32, tag="d")
                nc.scalar.activation(d_t[:], xc_t[:], Act.Copy,
                                     bias=4.0, scale=1.5)
                # u on DVE (exact round-to-nearest via magic constant)
                ub_t = du_pool.tile([P, FT], bf16, tag="ub")
                nc.vector._custom_dve(CR_FRAC, out=ub_t[:], in0=xr_t[:],
                                      s0=MAGIC, s1=2.0)
                # u^2 on ACT
                u2_t = du_pool.tile([P, FT], bf16, tag="u2")
                nc.scalar.activation(u2_t[:], ub_t[:], Act.Square)

                # H_t = T_t[:,1] + sum_{v=2..7} (d>=v) * D_t[:,v-2]
                # t=0..2: 3 custom insts per t, interleaved round-robin so
                # consecutive DVE insts avoid back-to-back RAW.
                # t=3 enters the output additively, so it is never
                # materialized: each of its six steps is an ACT sign-step
                #   c_v = (D_v/2)*sign(d-v) + (D_v/2)   [base folded in c_2]
                # and PE sums the six pieces straight into PSUM.
                hf = [acc_pool.tile([P, FT], f32, tag=f"h{t}", name=f"h{t}")
                      for t in range(3)]
                hb = [hb_pool.tile([P, FT], bf16, tag=f"hb{t}", name=f"hb{t}")
                      for t in range(3)]
                sg = [hb_pool.tile([P, FT], bf16, tag=f"sg{v}", name=f"sg{v}")
                      for v in range(6)]
                for t in range(3):
                    nc.vector._custom_dve(
                        CR_INIT2, out=hf[t][:], in0=d_t[:], in1=D[t][:, 1:2],
                        s0=D[t][:, 0:1], s1=T[t][:, 0:1], imm2=2.0)
                for v in range(6):
                    nc.scalar.activation(sg[v][:], d_t[:], Act.Sign,
                                         bias=cmv[:, v:v + 1])
                for t in range(3):
                    nc.vector._custom_dve(
                        CR_ACC2, out=hf[t][:], in0=d_t[:], in1=hf[t][:],
                        s0=D[t][:, 2:3], s1=D[t][:, 3:4], imm2=4.0)
                for t in range(3):
                    nc.vector._custom_dve(
                        CR_ACC2, out=hb[t][:], in0=d_t[:], in1=hf[t][:],
                        s0=D[t][:, 4:5], s1=D[t][:, 5:6], imm2=6.0)

                # powers-form products, bf16 2x-mode TT on DVE:
                #   m0 = (h0*u^2)*u, m1 = h1*u^2, m2 = h2*u
                t0 = m_pool.tile([P, FT], bf16, tag="t0")
                m0 = m_pool.tile([P, FT], bf16, tag="m0")
                m1 = m_pool.tile([P, FT], bf16, tag="m1")
                m2 = m_pool.tile([P, FT], bf16, tag="m2")
                nc.vector.tensor_tensor(t0[:], hb[0][:], u2_t[:], Alu.mult)
                nc.vector.tensor_tensor(m0[:], t0[:], ub_t[:], Alu.mult)
                nc.vector.tensor_tensor(m1[:], hb[1][:], u2_t[:], Alu.mult)
                nc.vector.tensor_tensor(m2[:], hb[2][:], ub_t[:], Alu.mult)

                # final sum on PE:
                #   psum = m0 + m1 + m2 + sum_v diag(D3_v/2) @ sign(d-v)
                ps_t = ps_pool.tile([P, FT], f32, tag="ps")
                for c in range(NCH):
                    cs = slice(c * 512, (c + 1) * 512)
                    nc.tensor.matmul(ps_t[:, cs], eye_t[:], m0[:, cs],
                                     start=True, stop=False)
                    nc.tensor.matmul(ps_t[:, cs], eye_t[:], m1[:, cs],
                                     start=False, stop=False)
                    nc.tensor.matmul(ps_t[:, cs], eye_t[:], m2[:, cs],
                                     start=False, stop=False)
                    for v in range(6):
                        nc.tensor.matmul(ps_t[:, cs], dg3[v][:], sg[v][:, cs],
                                         start=False, stop=(v == 5))

                # PSUM -> SBUF on ACT (adds the collapsed chain-3 bias)
                o_t = out_pool.tile([P, FT], f32, tag="o")
                nc.scalar.activation(o_t[:], ps_t[:], Act.Identity,
                                     bias=b3t[:, 0:1])
                nc.sync.dma_start(out[psl, fsl], o_t[:])

    nc.finalize()
    return nc


def _get_nc():
    if "nc" not in _CACHE:
        _CACHE["nc"] = _build_bass()
    return _CACHE["nc"]


def build_in_maps(x: np.ndarray, cp: np.ndarray) -> list[dict]:
    mrows = M // NCORES  # 1024 output rows per core
    eye = np.eye(P, dtype=ml_dtypes.bfloat16)
    in_maps = []
    for c in range(NCORES):
        xr = np.ascontiguousarray(
            x[c * mrows:(c + 1) * mrows, :]).reshape(NL, M)
        xct = np.ascontiguousarray(x[:, c * NL:(c + 1) * NL].T)
        cpc = np.ascontiguousarray(cp[c * NL:(c + 1) * NL, :])
        # diag(D3_v/2) weights for the PE sign-step path, per 128-row block
        dg = np.zeros((2 * 6, P, P), dtype=np.float32)
        for jb in range(2):
            blk = cpc[jb * P:(jb + 1) * P, :]
            for v in range(6):
                d3v = 0.5 * (blk[:, v + 3] - blk[:, v + 2])
                np.fill_diagonal(dg[jb * 6 + v], d3v)
        in_maps.append({"xr": xr, "xct": xct, "cp": cpc, "eye": eye,
                        "dg3": dg.astype(ml_dtypes.bfloat16)})
    return in_maps


def kernel(x: np.ndarray, control_points: np.ndarray) -> np.ndarray:
    x = np.ascontiguousarray(np.asarray(x, dtype=np.float32))
    cp = np.ascontiguousarray(np.asarray(control_points, dtype=np.float32))
    assert x.shape == (M, N) and cp.shape == (N, K)

    nc = _get_nc()
    mrows = M // NCORES  # 1024 output rows per core
    in_maps = build_in_maps(x, cp)

    res = run_bass_kernel_spmd(nc, in_maps, core_ids=list(range(NCORES)))
    outs = [res.results[c]["out"].reshape(mrows, N) for c in range(NCORES)]
    return np.concatenate(outs, axis=0)


# revision 27
# speedup vs baseline: 1.6388x; 1.0188x over previous
"""Catmull-Rom activation kernel for 8 TRN2 NeuronCores.

Reference semantics (m=8192 samples, n=2048 neurons, K=10 control points):
  p0    = floor(((x+2)*6)/4 + 1), clamped to 1 at x<=-2 and 7 at x>=2
  u     = frac(2x)
  coef  = (U @ B)[:, ::-1]   with U = [u^3, u^2, u, 1]   (sample-major flat)
  Q_k   = CP[j, p0+k-1]                                  (neuron-major flat)
  out   = sum_k coef_k * Q_k elementwise ON MISMATCHED FLATTENS: at flat
          position p (sample-major), coef comes from x[p//n, p%n] while Q
          comes from neuron jq=p//m, sample iq=p%m.

Algebraic regrouping: out = H0*u^3 + H1*u^2 + H2*u + H3 where
  H_t = T_t[j, s],  T_t[j, v] = sum_k B[t, 3-k] * CP[j, v+k-1],  s in 1..7.

Per-core layout (core c of 8): all work happens in the neuron-major
"H layout" (256 neurons x 8192 samples). Each core receives
  xr  = x[1024c:1024(c+1), :].reshape(256, 8192)   (u side)
  xct = x[:, 256c:256(c+1)].T                       (segment side)
  cp  = control_points[256c:256(c+1), :]
and its (256, 8192) output block is exactly out rows [1024c, 1024(c+1))
reinterpreted. No collectives, no on-device transposes.

Engine assignment (GpSimd/Pool is deliberately IDLE: it shares an SBUF
port with the DVE and the two serialize, so any Pool op costs more port
time than the same op on DVE):
  Scalar/ACT (own SBUF port): d = 1.5*x+4 (segment line, steps saturate
      so no clamp needed); u^2 = Square(u); PSUM->SBUF output copy.
  Vector/DVE: u = frac(2x) via magic-rounding custom op (bf16 out);
      4 H_t step-accumulation chains (3 custom insts each, fp32,
      final inst writes bf16); 4 bf16 2x-mode tensor_tensor multiplies
      (h0*u^2, (h0*u^2)*u, h1*u^2, h2*u).
  Tensor/PE: final sum m0+m1+m2+h3 via identity-weight matmuls
      accumulated in PSUM (bf16 inputs, fp32 accumulate).
"""

import sys

import numpy as np

sys.path.insert(0, "/opt/trn_rl_repo")

from contextlib import ExitStack

import concourse.bass as bass
import concourse.bacc as bacc
import concourse.mybir as mybir
from concourse import tile
from concourse import dve_ops
from concourse.dve_spec import (
    Spec, Src0, Src1, C0, C1, C2, Zero, One, maxx, minn, lower, _has_src1,
)
from concourse.dve_uop import DveOpSpec
from concourse.bass_utils import run_bass_kernel_spmd

import ml_dtypes

M = 8192          # samples
N = 2048          # neurons
K = 10            # control points per neuron
NCORES = 8
NL = N // NCORES  # 256 neurons per core
P = 128           # partitions per tile
FT = 2048         # free-dim tile size
f32 = mybir.dt.float32
bf16 = mybir.dt.bfloat16
Alu = mybir.AluOpType
Act = mybir.ActivationFunctionType
MAGIC = 12582912.0  # 1.5 * 2^23: rne-to-integer bias, valid for |t| < 2^22

# Wrev[t, k] = B[t, 3-k]; T_t[:, v] = sum_k Wrev[t,k] * CP[:, v-1+k]
_B = 0.5 * np.array(
    [[-1.0, 3.0, -3.0, 1.0],
     [2.0, -5.0, 4.0, -1.0],
     [-1.0, 0.0, 1.0, 0.0],
     [0.0, 2.0, 0.0, 0.0]], dtype=np.float32)
WREV = np.ascontiguousarray(_B[:, ::-1])  # (4, 4)

_CACHE = {}


def _register_op(name, spec):
    for o in dve_ops.OPS:
        if o.name == name:
            return o
    row = max(dve_ops._SUB_OPCODE_FOR_NAME.values()) + 1
    assert row < 0x20
    dve_ops._SUB_OPCODE_FOR_NAME[name] = row
    shas = {}
    for ver in ("v3", "v4"):
        u = lower(spec, ver=ver)
        shas[ver] = DveOpSpec(
            name=name, opcode=row, uops=u, rd1_en=_has_src1(spec)).sha(ver)
    op = dve_ops.DveOp(name, spec, subdim=False, uops_sha=shas)
    dve_ops.OPS.append(op)
    dve_ops.CUSTOM_DVE_SPECS[name] = spec
    return op


def _frac_ref(in0, in1, s0, s1, imm2):
    a = (np.asarray(in0, np.float32) * np.float32(s1)).astype(np.float32)
    r = ((a + np.float32(s0)).astype(np.float32) - np.float32(s0)).astype(np.float32)
    w = (a - r).astype(np.float32)
    return (w + (w < 0).astype(np.float32)).astype(np.float32)


def _acc2_ref(in0, in1, s0, s1, imm2):
    d = np.asarray(in0, np.float32)
    m0 = (d >= np.float32(imm2)).astype(np.float32)
    m1 = (d >= np.float32(imm2) + np.float32(1.0)).astype(np.float32)
    return (np.asarray(in1, np.float32) + m0 * np.asarray(s0, np.float32)
            + m1 * np.asarray(s1, np.float32)).astype(np.float32)


_a = Src0 * C1
_w = _a - ((_a + C0) - C0)
CR_FRAC = _register_op("CR_FRAC_ANT", Spec(
    body=_w + (_w < Zero),
    reference=_frac_ref))

CR_ACC2 = _register_op("CR_ACC2_ANT", Spec(
    body=Src1 + (Src0 >= C2) * C0 + (Src0 >= (C2 + One)) * C1,
    reference=_acc2_ref))


def _init2_ref(in0, in1, s0, s1, imm2):
    # in1 is the C3-spilled [P,1] scalar (second delta); s1 = base
    d = np.asarray(in0, np.float32)
    m0 = (d >= np.float32(imm2)).astype(np.float32)
    m1 = (d >= np.float32(imm2) + np.float32(1.0)).astype(np.float32)
    return (np.asarray(s1, np.float32) + m0 * np.asarray(s0, np.float32)
            + m1 * np.asarray(in1, np.float32)).astype(np.float32)


from concourse.dve_spec import C3, _spill_c3_to_src1  # noqa: E402

CR_INIT2 = _register_op("CR_INIT2_ANT", Spec(
    body=_spill_c3_to_src1(
        C1 + (Src0 >= C2) * C0 + (Src0 >= (C2 + One)) * C3),
    reference=_init2_ref))


def _ss2_ref(in0, in1, s0, s1, imm2):
    # two step-scaled terms, single-src
    d = np.asarray(in0, np.float32)
    m0 = (d >= np.float32(imm2)).astype(np.float32)
    m1 = (d >= np.float32(imm2) + np.float32(1.0)).astype(np.float32)
    return (m0 * np.asarray(s0, np.float32)
            + m1 * np.asarray(s1, np.float32)).astype(np.float32)


CR_SS2 = _register_op("CR_SS2_ANT", Spec(
    body=(Src0 >= C2) * C0 + (Src0 >= (C2 + One)) * C1,
    reference=_ss2_ref))


def _build_bass(gens: int = 1):
    nc = bacc.Bacc("TRN2", target_bir_lowering=False, debug=False,
                   num_devices=NCORES)
    xr = nc.dram_tensor("xr", [NL, M], f32, kind="ExternalInput").ap()
    xct = nc.dram_tensor("xct", [NL, M], f32, kind="ExternalInput").ap()
    cp = nc.dram_tensor("cp", [NL, K], f32, kind="ExternalInput").ap()
    eye = nc.dram_tensor("eye", [P, P], bf16, kind="ExternalInput").ap()
    dgt = nc.dram_tensor("dg3", [2 * 6, P, P], bf16, kind="ExternalInput").ap()
    out = nc.dram_tensor("out", [NL, M], f32, kind="ExternalOutput").ap()

    NCH = FT // 512  # PSUM matmul chunks per tile

    with tile.TileContext(nc, num_cores=NCORES) as tc, ExitStack() as ctx:
        const_pool = ctx.enter_context(tc.tile_pool(name="const", bufs=1))
        in_pool = ctx.enter_context(tc.tile_pool(name="inp", bufs=2))
        du_pool = ctx.enter_context(tc.tile_pool(name="du", bufs=2))
        acc_pool = ctx.enter_context(tc.tile_pool(name="acc", bufs=1))
        hb_pool = ctx.enter_context(tc.tile_pool(name="hb", bufs=2))
        m_pool = ctx.enter_context(tc.tile_pool(name="m", bufs=1))
        sg_pool = ctx.enter_context(tc.tile_pool(name="sg", bufs=1))
        out_pool = ctx.enter_context(tc.tile_pool(name="outp", bufs=2))
        ps_pool = ctx.enter_context(
            tc.tile_pool(name="ps", bufs=2, space="PSUM"))

        eye_t = const_pool.tile([P, P], bf16, tag="eye")
        nc.sync.dma_start(eye_t[:], eye[:, :])

        for jb in range(NL // P):  # two 128-neuron blocks
            # ---- tiny per-block table prep (on DVE; negligible) ----
            cpt = const_pool.tile([P, K], f32, tag=f"cp{jb}")
            nc.sync.dma_start(cpt[:], cp[jb * P:(jb + 1) * P, :])
            # T_t[:, vi] for vi=0..6 (v=vi+1); D_t[:, vi] = T(vi+1)-T(vi)
            T = [const_pool.tile([P, 7], f32, tag=f"T{jb}_{t}", name=f"T{jb}_{t}")
                 for t in range(4)]
            D = [const_pool.tile([P, 6], f32, tag=f"D{jb}_{t}", name=f"D{jb}_{t}")
                 for t in range(4)]
            for t in range(4):
                nc.vector.tensor_single_scalar(
                    T[t][:], cpt[:, 0:7], float(WREV[t, 0]), Alu.mult)
                for k in range(1, 4):
                    nc.vector.scalar_tensor_tensor(
                        T[t][:], cpt[:, k:k + 7], float(WREV[t, k]), T[t][:],
                        Alu.mult, Alu.add)
                nc.vector.tensor_sub(D[t][:], T[t][:, 1:7], T[t][:, 0:6])
            # chain 3 runs as ACT sign-steps + PE diagonal-weight scaling:
            # psum += diag(D3_v/2) @ sign(d-v); all the +D/2 offsets and the
            # base collapse to bias3 = 0.5*(cp[:,2]+cp[:,8]) applied in the
            # final PSUM->SBUF copy. diag weights are shipped from the host.
            b3t = const_pool.tile([P, 1], f32, tag=f"b3t{jb}",
                                  name=f"b3t{jb}")
            nc.vector.tensor_add(b3t[:], cpt[:, 2:3], cpt[:, 8:9])
            nc.vector.tensor_single_scalar(b3t[:], b3t[:], 0.5, Alu.mult)
            dg3 = [const_pool.tile([P, P], bf16, tag=f"dg{jb}_{v}",
                                   name=f"dg{jb}_{v}") for v in range(6)]
            for v in range(6):
                nc.sync.dma_start(dg3[v][:], dgt[jb * 6 + v, :, :])
            cmv = const_pool.tile([P, 6], f32, tag=f"cmv{jb}",
                                  name=f"cmv{jb}")
            for v in range(6):
                nc.vector.memset(cmv[:, v:v + 1], -2.0 - v)

            for fc in range(M // FT):
                fsl = slice(fc * FT, (fc + 1) * FT)
                psl = slice(jb * P, (jb + 1) * P)

                xr_t = in_pool.tile([P, FT], f32, tag="xr")
                nc.sync.dma_start(xr_t[:], xr[psl, fsl])
                xc_t = in_pool.tile([P, FT], f32, tag="xc")
                nc.sync.dma_start(xc_t[:], xct[psl, fsl])

                # segment line on ACT: d = 1.5*x + 4  (steps saturate, so
                # the reference's clamp at the range edges is automatic)
                d_t = du_pool.tile([P, FT], f32, tag="d")
                nc.scalar.activation(d_t[:], xc_t[:], Act.Copy,
                                     bias=4.0, scale=1.5)
                # u on DVE (exact round-to-nearest via magic constant)
                ub_t = du_pool.tile([P, FT], bf16, tag="ub")
                nc.vector._custom_dve(CR_FRAC, out=ub_t[:], in0=xr_t[:],
                                      s0=MAGIC, s1=2.0)
                # u^2 on ACT
                u2_t = du_pool.tile([P, FT], bf16, tag="u2")
                nc.scalar.activation(u2_t[:], ub_t[:], Act.Square)

                # H_t = T_t[:,1] + sum_{v=2..7} (d>=v) * D_t[:,v-2]
                # t=0..2: 3 custom insts per t, interleaved round-robin so
                # consecutive DVE insts avoid back-to-back RAW.
                # t=3 enters the output additively, so it is never
                # materialized: each of its six steps is an ACT sign-step
                #   c_v = (D_v/2)*sign(d-v) + (D_v/2)   [base folded in c_2]
                # and PE sums the six pieces straight into PSUM.
                hf = [acc_pool.tile([P, FT], f32, tag=f"h{t}", name=f"h{t}")
                      for t in range(3)]
                hb = [hb_pool.tile([P, FT], bf16, tag=f"hb{t}", name=f"hb{t}")
                      for t in range(3)]
                sg = [hb_pool.tile([P, FT], bf16, tag=f"sg{v}", name=f"sg{v}")
                      for v in range(6)]
                for t in range(3):
                    nc.vector._custom_dve(
                        CR_INIT2, out=hf[t][:], in0=d_t[:], in1=D[t][:, 1:2],
                        s0=D[t][:, 0:1], s1=T[t][:, 0:1], imm2=2.0)
                for v in range(6):
                    nc.scalar.activation(sg[v][:], d_t[:], Act.Sign,
                                         bias=cmv[:, v:v + 1])
                for t in range(3):
                    nc.vector._custom_dve(
                        CR_ACC2, out=hf[t][:], in0=d_t[:], in1=hf[t][:],
                        s0=D[t][:, 2:3], s1=D[t][:, 3:4], imm2=4.0)
                for t in range(3):
                    nc.vector._custom_dve(
                        CR_ACC2, out=hb[t][:], in0=d_t[:], in1=hf[t][:],
                        s0=D[t][:, 4:5], s1=D[t][:, 5:6], imm2=6.0)

                # u^3 on ACT via exp(3*ln(u));  ln(0) -> -inf -> exp -> 0
                ul_t = m_pool.tile([P, FT], f32, tag="ul")
                nc.scalar.activation(ul_t[:], ub_t[:], Act.Ln)
                u3_t = m_pool.tile([P, FT], bf16, tag="u3")
                nc.scalar.activation(u3_t[:], ul_t[:], Act.Exp, scale=3.0)

                # powers-form products, bf16 2x-mode TT on DVE:
                #   m0 = h0*u^3, m1 = h1*u^2, m2 = h2*u
                m0 = m_pool.tile([P, FT], bf16, tag="m0")
                m1 = m_pool.tile([P, FT], bf16, tag="m1")
                m2 = m_pool.tile([P, FT], bf16, tag="m2")
                nc.vector.tensor_tensor(m1[:], hb[1][:], u2_t[:], Alu.mult)
                nc.vector.tensor_tensor(m0[:], hb[0][:], u3_t[:], Alu.mult)
                nc.vector.tensor_tensor(m2[:], hb[2][:], ub_t[:], Alu.mult)

                # final sum on PE, early-ready sign tensors first:
                #   psum = sum_v diag(D3_v/2) @ sign(d-v) + m0 + m1 + m2
                ps_t = ps_pool.tile([P, FT], f32, tag="ps")
                o_t = out_pool.tile([P, FT], f32, tag="o")
                for c in range(NCH):
                    cs = slice(c * 512, (c + 1) * 512)
                    for v in range(6):
                        nc.tensor.matmul(ps_t[:, cs], dg3[v][:], sg[v][:, cs],
                                         start=(v == 0), stop=False)
                    nc.tensor.matmul(ps_t[:, cs], eye_t[:], m1[:, cs],
                                     start=False, stop=False)
                    nc.tensor.matmul(ps_t[:, cs], eye_t[:], m0[:, cs],
                                     start=False, stop=False)
                    nc.tensor.matmul(ps_t[:, cs], eye_t[:], m2[:, cs],
                                     start=False, stop=True)
                    # PSUM -> SBUF per chunk on ACT (adds collapsed c3 bias)
                    nc.scalar.activation(o_t[:, cs], ps_t[:, cs],
                                         Act.Identity, bias=b3t[:, 0:1])
                    nc.sync.dma_start(out[psl, fsl][:, cs], o_t[:, cs])

    nc.finalize()
    return nc


def _get_nc():
    if "nc" not in _CACHE:
        _CACHE["nc"] = _build_bass()
    return _CACHE["nc"]


def build_in_maps(x: np.ndarray, cp: np.ndarray) -> list[dict]:
    mrows = M // NCORES  # 1024 output rows per core
    eye = np.eye(P, dtype=ml_dtypes.bfloat16)
    in_maps = []
    for c in range(NCORES):
        xr = np.ascontiguousarray(
            x[c * mrows:(c + 1) * mrows, :]).reshape(NL, M)
        xct = np.ascontiguousarray(x[:, c * NL:(c + 1) * NL].T)
        cpc = np.ascontiguousarray(cp[c * NL:(c + 1) * NL, :])
        # diag(D3_v/2) weights for the PE sign-step path, per 128-row block
        dg = np.zeros((2 * 6, P, P), dtype=np.float32)
        for jb in range(2):
            blk = cpc[jb * P:(jb + 1) * P, :]
            for v in range(6):
                d3v = 0.5 * (blk[:, v + 3] - blk[:, v + 2])
                np.fill_diagonal(dg[jb * 6 + v], d3v)
        in_maps.append({"xr": xr, "xct": xct, "cp": cpc, "eye": eye,
                        "dg3": dg.astype(ml_dtypes.bfloat16)})
    return in_maps


def kernel(x: np.ndarray, control_points: np.ndarray) -> np.ndarray:
    x = np.ascontiguousarray(np.asarray(x, dtype=np.float32))
    cp = np.ascontiguousarray(np.asarray(control_points, dtype=np.float32))
    assert x.shape == (M, N) and cp.shape == (N, K)

    nc = _get_nc()
    mrows = M // NCORES  # 1024 output rows per core
    in_maps = build_in_maps(x, cp)

    res = run_bass_kernel_spmd(nc, in_maps, core_ids=list(range(NCORES)))
    outs = [res.results[c]["out"].reshape(mrows, N) for c in range(NCORES)]
    return np.concatenate(outs, axis=0)
